# revision 27
# baseline (speedup 1.0000x reference)
"""Trainium2 Bass kernel for nn_AddInterpolant — v2 (host-transposed I/O).

Same math as kernel.py, but the host pre-transposes x0/x1 per shard and
post-transposes the outputs, so the on-chip kernel is pure matmuls: no
PE transposes, no identity, and the interpolation combine runs in the
transposed (feature-major) layout using a broadcast-t row built with one
rank-1 fp32 matmul per stripe.
"""

import sys

for _p in ("/opt/trn_rl_repo",):
    if _p not in sys.path:
        sys.path.insert(0, _p)

import numpy as np

import concourse.mybir as mybir
import concourse.tile as tile
from concourse import bacc
from concourse.bass import ds
from concourse.bass_utils import run_bass_kernel_spmd

P = 128
D = 256  # state dim
H = 1024  # hidden dim
B = 65536  # global batch
NCORES = 8
BL = B // NCORES  # rows per core
S = 512  # batch columns per stripe
NSTRIPES = BL // S
HC = H // P  # 8 hidden chunks
DC = D // P  # 2 state chunks

F32 = mybir.dt.float32
BF16 = mybir.dt.bfloat16
RELU = mybir.ActivationFunctionType.Relu
IDENT = mybir.ActivationFunctionType.Identity
SIGN = mybir.ActivationFunctionType.Sign
GT = mybir.AluOpType.is_gt
MULT = mybir.AluOpType.mult
ADD = mybir.AluOpType.add
SUB = mybir.AluOpType.subtract
MAX = mybir.AluOpType.max

_nc_cache = None


def build():
    nc = bacc.Bacc(None)

    x0Te = nc.declare_dram_parameter("x0T", [D, BL], F32, isOutput=False)
    x1Te = nc.declare_dram_parameter("x1T", [D, BL], F32, isOutput=False)
    te = nc.declare_dram_parameter("t", [BL, 1], F32, isOutput=False)
    W1e = nc.declare_dram_parameter("W1b", [2 * D + 1, H], BF16, isOutput=False)
    w1re = nc.declare_dram_parameter("w1row", [H], F32, isOutput=False)
    b1e = nc.declare_dram_parameter("b1", [H], F32, isOutput=False)
    W2e = nc.declare_dram_parameter("W2b", [H, H], BF16, isOutput=False)
    b2e = nc.declare_dram_parameter("b2", [H], F32, isOutput=False)
    W3e = nc.declare_dram_parameter("W3b", [H, H], BF16, isOutput=False)
    b3e = nc.declare_dram_parameter("b3", [H], F32, isOutput=False)
    W4e = nc.declare_dram_parameter("W4b", [H, D], BF16, isOutput=False)
    w15e = nc.declare_dram_parameter("w15b", [P, H], BF16, isOutput=False)
    z5e = nc.declare_dram_parameter("z5z", [P, S], BF16, isOutput=False)
    b4e = nc.declare_dram_parameter("b4", [D], F32, isOutput=False)
    xtTe = nc.declare_dram_parameter("xtT", [D, BL], F32, isOutput=True)
    dtTe = nc.declare_dram_parameter("dtT", [D, BL], F32, isOutput=True)

    x0Tv = x0Te.rearrange("(c p) b -> p c b", p=P)
    x1Tv = x1Te.rearrange("(c p) b -> p c b", p=P)
    xtTv = xtTe.rearrange("(c p) b -> p c b", p=P)
    dtTv = dtTe.rearrange("(c p) b -> p c b", p=P)

    with tile.TileContext(nc) as tc:
        with (
            tc.tile_pool(name="const", bufs=1) as cp,
            tc.tile_pool(name="z", bufs=1) as zp,
            tc.tile_pool(name="acts", bufs=1) as hp,
            tc.tile_pool(name="outs", bufs=1) as fp,
            tc.tile_pool(name="nat", bufs=2) as npl,
            tc.tile_pool(name="small", bufs=2) as sp,
            tc.tile_pool(name="mm", bufs=2, space="PSUM") as mmp,
        ):
            def emit_input(s):
                row0 = s * S
                ztf0 = npl.tile([P, DC, S], F32, tag="ztf0", name=f"ztf0_{s}")
                nc.sync.dma_start(ztf0[:], x0Tv[:, :, ds(row0, S)])
                ztf1 = npl.tile([P, DC, S], F32, tag="ztf1", name=f"ztf1_{s}")
                nc.sync.dma_start(ztf1[:], x1Tv[:, :, ds(row0, S)])
                trowst = sp.tile([1, S], F32, tag="trowst", name=f"trowst_{s}")
                nc.sync.dma_start(
                    trowst[:], te[ds(row0, S), 0:1].rearrange("b one -> one b")
                )
                return ztf0, ztf1, trowst

            pending = emit_input(0)
            # ---- weights arrive bf16 from the host: direct DMA ----
            w1s = cp.tile([P, 4, H], BF16)
            nc.sync.dma_start(
                w1s[:], W1e[0 : 2 * D].rearrange("(o p) n -> p o n", p=P)
            )
            w2s = cp.tile([P, HC, H], BF16)
            nc.sync.dma_start(w2s[:], W2e.rearrange("(o p) n -> p o n", p=P))
            w3s = cp.tile([P, HC, H], BF16)
            nc.sync.dma_start(w3s[:], W3e.rearrange("(o p) n -> p o n", p=P))
            w4s = cp.tile([P, HC, D], BF16)
            nc.sync.dma_start(w4s[:], W4e.rearrange("(o p) n -> p o n", p=P))

            w1rp = cp.tile([P, HC], F32)
            nc.sync.dma_start(w1rp[:], w1re.rearrange("(o p) -> p o", p=P))
            b1p = cp.tile([P, HC], F32)
            nc.sync.dma_start(b1p[:], b1e.rearrange("(o p) -> p o", p=P))
            b2p = cp.tile([P, HC], F32)
            nc.sync.dma_start(b2p[:], b2e.rearrange("(o p) -> p o", p=P))
            b3p = cp.tile([P, HC], F32)
            nc.sync.dma_start(b3p[:], b3e.rearrange("(o p) -> p o", p=P))
            b4p = cp.tile([P, DC], F32)
            nc.sync.dma_start(b4p[:], b4e.rearrange("(o p) -> p o", p=P))

            # padded "t chunk": Z5 row0 = t (per stripe), rest 0; W15 row0 = W1[512]
            z5 = cp.tile([P, S], BF16)
            nc.sync.dma_start(z5[:], z5e[:])
            w15 = cp.tile([P, H], BF16)
            nc.sync.dma_start(w15[:], w15e[:])


            for s in range(NSTRIPES):
                row0 = s * S
                ztf0, ztf1, trowst = pending
                nc.vector.tensor_copy(z5[0:1, :], trowst[:])

                # cast inputs to bf16 zT chunks
                zT = zp.tile([P, 4, S], BF16, tag="zT")
                nc.vector.tensor_copy(zT[:, 0:2, :], ztf0[:])
                nc.vector.tensor_copy(zT[:, 2:4, :], ztf1[:])

                # broadcast t to all partitions with a stride-0 DMA read
                tsb = sp.tile([P, S], F32, tag="tsb")
                nc.sync.dma_start(
                    tsb[:],
                    te[ds(row0, S), 0:1]
                    .rearrange("b one -> one b")
                    .to_broadcast((P, S)),
                )
                omt = sp.tile([P, S], F32, tag="omt")
                nc.vector.tensor_scalar(omt[:], tsb[:], -1.0, 1.0, MULT, ADD)
                om2t = sp.tile([P, S], F32, tag="om2t")
                nc.vector.tensor_scalar(om2t[:], tsb[:], -2.0, 1.0, MULT, ADD)
                a_ = sp.tile([P, S], F32, tag="a_")
                nc.vector.tensor_tensor(a_[:], tsb[:], tsb[:], MULT)
                nc.vector.tensor_tensor(a_[:], tsb[:], a_[:], SUB)

                # ---- layer 1 ----
                h1 = hp.tile([P, HC, S], BF16, tag="hA")
                dh1 = hp.tile([P, HC, S], BF16, tag="dhA")
                for m in range(HC):
                    psf = mmp.tile([P, S], F32, tag="mmf")
                    for k in range(4):
                        nc.tensor.matmul(
                            psf[:],
                            w1s[:, k, ds(m * P, P)],
                            zT[:, k, :],
                            start=(k == 0),
                            stop=False,
                        )
                    nc.tensor.matmul(
                        psf[:], w15[:, ds(m * P, P)], z5[:], start=False, stop=True
                    )
                    nc.scalar.activation(
                        h1[:, m, :], psf[:], RELU, bias=b1p[:, m : m + 1]
                    )
                    nc.vector.tensor_scalar(
                        dh1[:, m, :], h1[:, m, :], 0.0, w1rp[:, m : m + 1], GT, MULT
                    )

                # ---- layers 2 and 3 ----
                hprev, dhprev = h1, dh1
                for li, (ws, bp) in enumerate(((w2s, b2p), (w3s, b3p))):
                    hn = hp.tile([P, HC, S], BF16, tag="hB" if li == 0 else "hA")
                    dhn = hp.tile([P, HC, S], BF16, tag="dhB" if li == 0 else "dhA")
                    for m in range(HC):
                        psf = mmp.tile([P, S], F32, tag="mmf")
                        pst = mmp.tile([P, S], F32, tag="mmt", bufs=4)
                        for k in range(HC):
                            nc.tensor.matmul(
                                psf[:],
                                ws[:, k, ds(m * P, P)],
                                hprev[:, k, :],
                                start=(k == 0),
                                stop=(k == HC - 1),
                            )
                            nc.tensor.matmul(
                                pst[:],
                                ws[:, k, ds(m * P, P)],
                                dhprev[:, k, :],
                                start=(k == 0),
                                stop=(k == HC - 1),
                            )
                        nc.vector.tensor_scalar(
                            hn[:, m, :], psf[:], bp[:, m : m + 1], 0.0, ADD, MAX
                        )
                        msk = sp.tile([P, S], F32, tag="mask", bufs=1)
                        nc.scalar.activation(msk[:], hn[:, m, :], SIGN)
                        nc.vector.tensor_tensor(dhn[:, m, :], msk[:], pst[:], MULT)
                    hprev, dhprev = hn, dhn

                # ---- layer 4 (no relu), f32 outputs for the combine ----
                fnnT = fp.tile([P, DC, S], F32, tag="fnnT")
                dfnnT = fp.tile([P, DC, S], F32, tag="dfnnT")
                for m in range(DC):
                    psf = mmp.tile([P, S], F32, tag="mmf")
                    pst = mmp.tile([P, S], F32, tag="mmt", bufs=4)
                    for k in range(HC):
                        nc.tensor.matmul(
                            psf[:],
                            w4s[:, k, ds(m * P, P)],
                            hprev[:, k, :],
                            start=(k == 0),
                            stop=(k == HC - 1),
                        )
                        nc.tensor.matmul(
                            pst[:],
                            w4s[:, k, ds(m * P, P)],
                            dhprev[:, k, :],
                            start=(k == 0),
                            stop=(k == HC - 1),
                        )
                    nc.scalar.activation(
                        fnnT[:, m, :], psf[:], IDENT, bias=b4p[:, m : m + 1]
                    )
                    nc.scalar.copy(dfnnT[:, m, :], pst[:])

                if s + 1 < NSTRIPES:
                    pending = emit_input(s + 1)

                # ---- combine in transposed space ----
                xtT = fp.tile([P, DC, S], F32, tag="xtT")
                dtT = fp.tile([P, DC, S], F32, tag="dtT")
                for fc in range(DC):
                    tm = sp.tile([P, S], F32, tag="tmp")
                    # dt = x1 - x0 first (uses raw f32 inputs)
                    nc.vector.tensor_tensor(
                        dtT[:, fc, :], ztf1[:, fc, :], ztf0[:, fc, :], SUB
                    )
                    nc.vector.tensor_tensor(
                        xtT[:, fc, :], ztf0[:, fc, :], omt[:], MULT
                    )
                    nc.vector.tensor_tensor(tm[:], ztf1[:, fc, :], tsb[:], MULT)
                    nc.vector.tensor_tensor(xtT[:, fc, :], xtT[:, fc, :], tm[:], ADD)
                    tm2 = sp.tile([P, S], F32, tag="tmp")
                    nc.vector.tensor_tensor(tm2[:], fnnT[:, fc, :], a_[:], MULT)
                    nc.vector.tensor_tensor(xtT[:, fc, :], xtT[:, fc, :], tm2[:], ADD)
                    tm3 = sp.tile([P, S], F32, tag="tmp")
                    nc.vector.tensor_tensor(tm3[:], fnnT[:, fc, :], om2t[:], MULT)
                    nc.vector.tensor_tensor(dtT[:, fc, :], dtT[:, fc, :], tm3[:], ADD)
                    tm4 = sp.tile([P, S], F32, tag="tmp")
                    nc.vector.tensor_tensor(tm4[:], dfnnT[:, fc, :], a_[:], MULT)
                    nc.vector.tensor_tensor(dtT[:, fc, :], dtT[:, fc, :], tm4[:], ADD)

                nc.sync.dma_start(xtTv[:, :, ds(row0, S)], xtT[:])
                nc.sync.dma_start(dtTv[:, :, ds(row0, S)], dtT[:])

    nc.compile()
    return nc


def _get_nc():
    global _nc_cache
    if _nc_cache is None:
        _nc_cache = build()
    return _nc_cache


def kernel(x0, x1, t, W1, b1, W2, b2, W3, b3, W4, b4, trace=False, **trace_kwargs):
    nc = _get_nc()
    import ml_dtypes

    bf = ml_dtypes.bfloat16
    W1 = np.asarray(W1, np.float32)
    reps = {
        "W1b": np.ascontiguousarray(W1.astype(bf)),
        "w1row": np.ascontiguousarray(W1[2 * D], np.float32),
        "b1": np.ascontiguousarray(b1, np.float32),
        "W2b": np.ascontiguousarray(np.asarray(W2, np.float32).astype(bf)),
        "b2": np.ascontiguousarray(b2, np.float32),
        "W3b": np.ascontiguousarray(np.asarray(W3, np.float32).astype(bf)),
        "b3": np.ascontiguousarray(b3, np.float32),
        "W4b": np.ascontiguousarray(np.asarray(W4, np.float32).astype(bf)),
        "b4": np.ascontiguousarray(b4, np.float32),
    }
    w15b = np.zeros((P, H), dtype=bf)
    w15b[0] = W1[2 * D].astype(bf)
    reps["w15b"] = w15b
    reps["z5z"] = np.zeros((P, S), dtype=bf)
    x0 = np.asarray(x0, np.float32)
    x1 = np.asarray(x1, np.float32)
    in_maps = []
    for c in range(NCORES):
        sl = slice(c * BL, (c + 1) * BL)
        in_maps.append(
            {
                "x0T": np.ascontiguousarray(x0[sl].T),
                "x1T": np.ascontiguousarray(x1[sl].T),
                "t": np.ascontiguousarray(t[sl], np.float32),
                **reps,
            }
        )
    res = run_bass_kernel_spmd(
        nc, in_maps, list(range(NCORES)), trace=trace, **trace_kwargs
    )
    xt = np.concatenate(
        [np.ascontiguousarray(res.results[c]["xtT"].T) for c in range(NCORES)], axis=0
    )
    dt_xt = np.concatenate(
        [np.ascontiguousarray(res.results[c]["dtT"].T) for c in range(NCORES)], axis=0
    )
    if trace:
        kernel.last_result = res
    return (xt, dt_xt)


# revision 28
# speedup vs baseline: 1.1935x; 1.1935x over previous
"""Trainium2 Bass kernel for nn_AddInterpolant — v2 (host-transposed I/O).

Same math as kernel.py, but the host pre-transposes x0/x1 per shard and
post-transposes the outputs, so the on-chip kernel is pure matmuls: no
PE transposes, no identity, and the interpolation combine runs in the
transposed (feature-major) layout using a broadcast-t row built with one
rank-1 fp32 matmul per stripe.
"""

import sys

for _p in ("/opt/trn_rl_repo",):
    if _p not in sys.path:
        sys.path.insert(0, _p)

import numpy as np

import concourse.mybir as mybir
import concourse.tile as tile
from concourse import bacc
from concourse.bass import ds
from concourse.bass_utils import run_bass_kernel_spmd

P = 128
D = 256  # state dim
H = 1024  # hidden dim
B = 65536  # global batch
NCORES = 8
BL = B // NCORES  # rows per core
S = 512  # batch columns per stripe
NSTRIPES = BL // S
HC = H // P  # 8 hidden chunks
DC = D // P  # 2 state chunks

F32 = mybir.dt.float32
BF16 = mybir.dt.bfloat16
RELU = mybir.ActivationFunctionType.Relu
IDENT = mybir.ActivationFunctionType.Identity
SIGN = mybir.ActivationFunctionType.Sign
GT = mybir.AluOpType.is_gt
MULT = mybir.AluOpType.mult
ADD = mybir.AluOpType.add
SUB = mybir.AluOpType.subtract
MAX = mybir.AluOpType.max

_nc_cache = None


def build():
    nc = bacc.Bacc(None)

    x0Te = nc.declare_dram_parameter("x0T", [D, BL], F32, isOutput=False)
    x1Te = nc.declare_dram_parameter("x1T", [D, BL], F32, isOutput=False)
    te = nc.declare_dram_parameter("t", [BL, 1], F32, isOutput=False)
    W1e = nc.declare_dram_parameter("W1b", [2 * D + 1, H], BF16, isOutput=False)
    w1re = nc.declare_dram_parameter("w1row", [H], F32, isOutput=False)
    b1e = nc.declare_dram_parameter("b1", [H], F32, isOutput=False)
    W2e = nc.declare_dram_parameter("W2b", [H, H], BF16, isOutput=False)
    b2e = nc.declare_dram_parameter("b2", [H], F32, isOutput=False)
    W3e = nc.declare_dram_parameter("W3b", [H, H], BF16, isOutput=False)
    b3e = nc.declare_dram_parameter("b3", [H], F32, isOutput=False)
    W4e = nc.declare_dram_parameter("W4b", [H, D], BF16, isOutput=False)
    w15e = nc.declare_dram_parameter("w15b", [P, H], BF16, isOutput=False)
    z5e = nc.declare_dram_parameter("z5z", [P, S], BF16, isOutput=False)
    b4e = nc.declare_dram_parameter("b4", [D], F32, isOutput=False)
    xtTe = nc.declare_dram_parameter("xtT", [D, BL], F32, isOutput=True)
    dtTe = nc.declare_dram_parameter("dtT", [D, BL], F32, isOutput=True)

    x0Tv = x0Te.rearrange("(c p) b -> p c b", p=P)
    x1Tv = x1Te.rearrange("(c p) b -> p c b", p=P)
    xtTv = xtTe.rearrange("(c p) b -> p c b", p=P)
    dtTv = dtTe.rearrange("(c p) b -> p c b", p=P)

    with tile.TileContext(nc) as tc:
        with (
            tc.tile_pool(name="const", bufs=1) as cp,
            tc.tile_pool(name="z", bufs=1) as zp,
            tc.tile_pool(name="acts", bufs=1) as hp,
            tc.tile_pool(name="outs", bufs=1) as fp,
            tc.tile_pool(name="nat", bufs=2) as npl,
            tc.tile_pool(name="small", bufs=2) as sp,
            tc.tile_pool(name="mm", bufs=2, space="PSUM") as mmp,
        ):
            def emit_input(s):
                row0 = s * S
                ztf0 = npl.tile([P, DC, S], F32, tag="ztf0", name=f"ztf0_{s}")
                nc.sync.dma_start(ztf0[:], x0Tv[:, :, ds(row0, S)])
                ztf1 = npl.tile([P, DC, S], F32, tag="ztf1", name=f"ztf1_{s}")
                nc.sync.dma_start(ztf1[:], x1Tv[:, :, ds(row0, S)])
                trowst = sp.tile([1, S], F32, tag="trowst", name=f"trowst_{s}")
                nc.sync.dma_start(
                    trowst[:], te[ds(row0, S), 0:1].rearrange("b one -> one b")
                )
                return ztf0, ztf1, trowst

            pending = emit_input(0)
            # ---- weights arrive bf16 from the host: direct DMA ----
            # everything layer 1 needs first, bulk W2/W3/W4 after
            w1s = cp.tile([P, 4, H], BF16)
            nc.sync.dma_start(
                w1s[:], W1e[0 : 2 * D].rearrange("(o p) n -> p o n", p=P)
            )
            z5 = cp.tile([P, S], BF16)
            nc.sync.dma_start(z5[:], z5e[:])
            w15 = cp.tile([P, H], BF16)
            nc.sync.dma_start(w15[:], w15e[:])
            w1rp = cp.tile([P, HC], F32)
            nc.sync.dma_start(w1rp[:], w1re.rearrange("(o p) -> p o", p=P))
            b1p = cp.tile([P, HC], F32)
            nc.sync.dma_start(b1p[:], b1e.rearrange("(o p) -> p o", p=P))

            w2s = cp.tile([P, HC, H], BF16)
            nc.sync.dma_start(w2s[:], W2e.rearrange("(o p) n -> p o n", p=P))
            w3s = cp.tile([P, HC, H], BF16)
            nc.sync.dma_start(w3s[:], W3e.rearrange("(o p) n -> p o n", p=P))
            w4s = cp.tile([P, HC, D], BF16)
            nc.sync.dma_start(w4s[:], W4e.rearrange("(o p) n -> p o n", p=P))
            b2p = cp.tile([P, HC], F32)
            nc.sync.dma_start(b2p[:], b2e.rearrange("(o p) -> p o", p=P))
            b3p = cp.tile([P, HC], F32)
            nc.sync.dma_start(b3p[:], b3e.rearrange("(o p) -> p o", p=P))
            b4p = cp.tile([P, DC], F32)
            nc.sync.dma_start(b4p[:], b4e.rearrange("(o p) -> p o", p=P))


            for s in range(NSTRIPES):
                row0 = s * S
                ztf0, ztf1, trowst = pending
                nc.vector.tensor_copy(z5[0:1, :], trowst[:])

                # cast inputs to bf16 zT chunks
                zT = zp.tile([P, 4, S], BF16, tag="zT")
                nc.vector.tensor_copy(zT[:, 0:2, :], ztf0[:])
                nc.vector.tensor_copy(zT[:, 2:4, :], ztf1[:])

                # broadcast t to all partitions with a stride-0 DMA read
                tsb = sp.tile([P, S], F32, tag="tsb")
                nc.sync.dma_start(
                    tsb[:],
                    te[ds(row0, S), 0:1]
                    .rearrange("b one -> one b")
                    .to_broadcast((P, S)),
                )
                omt = sp.tile([P, S], F32, tag="omt")
                nc.vector.tensor_scalar(omt[:], tsb[:], -1.0, 1.0, MULT, ADD)
                om2t = sp.tile([P, S], F32, tag="om2t")
                nc.vector.tensor_scalar(om2t[:], tsb[:], -2.0, 1.0, MULT, ADD)
                a_ = sp.tile([P, S], F32, tag="a_")
                nc.vector.tensor_tensor(a_[:], tsb[:], tsb[:], MULT)
                nc.vector.tensor_tensor(a_[:], tsb[:], a_[:], SUB)

                # ---- layer 1 ----
                h1 = hp.tile([P, HC, S], BF16, tag="hA")
                dh1 = hp.tile([P, HC, S], BF16, tag="dhA")
                for m in range(HC):
                    psf = mmp.tile([P, S], F32, tag="mmf")
                    for k in range(4):
                        nc.tensor.matmul(
                            psf[:],
                            w1s[:, k, ds(m * P, P)],
                            zT[:, k, :],
                            start=(k == 0),
                            stop=False,
                        )
                    nc.tensor.matmul(
                        psf[:], w15[:, ds(m * P, P)], z5[:], start=False, stop=True
                    )
                    nc.scalar.activation(
                        h1[:, m, :], psf[:], RELU, bias=b1p[:, m : m + 1]
                    )
                    nc.vector.tensor_scalar(
                        dh1[:, m, :], h1[:, m, :], 0.0, w1rp[:, m : m + 1], GT, MULT
                    )

                # ---- layers 2 and 3 ----
                hprev, dhprev = h1, dh1
                for li, (ws, bp) in enumerate(((w2s, b2p), (w3s, b3p))):
                    hn = hp.tile([P, HC, S], BF16, tag="hB" if li == 0 else "hA")
                    dhn = hp.tile([P, HC, S], BF16, tag="dhB" if li == 0 else "dhA")
                    for m in range(HC):
                        psf = mmp.tile([P, S], F32, tag="mmf")
                        pst = mmp.tile([P, S], F32, tag="mmt", bufs=4)
                        for k in range(HC):
                            nc.tensor.matmul(
                                psf[:],
                                ws[:, k, ds(m * P, P)],
                                hprev[:, k, :],
                                start=(k == 0),
                                stop=(k == HC - 1),
                            )
                            nc.tensor.matmul(
                                pst[:],
                                ws[:, k, ds(m * P, P)],
                                dhprev[:, k, :],
                                start=(k == 0),
                                stop=(k == HC - 1),
                            )
                        nc.vector.tensor_scalar(
                            hn[:, m, :], psf[:], bp[:, m : m + 1], 0.0, ADD, MAX
                        )
                        msk = sp.tile([P, S], F32, tag="mask", bufs=1)
                        nc.scalar.activation(msk[:], hn[:, m, :], SIGN)
                        nc.vector.tensor_tensor(dhn[:, m, :], msk[:], pst[:], MULT)
                    hprev, dhprev = hn, dhn

                # ---- layer 4 (no relu), f32 outputs for the combine ----
                fnnT = fp.tile([P, DC, S], F32, tag="fnnT")
                dfnnT = fp.tile([P, DC, S], F32, tag="dfnnT")
                for m in range(DC):
                    psf = mmp.tile([P, S], F32, tag="mmf")
                    pst = mmp.tile([P, S], F32, tag="mmt", bufs=4)
                    for k in range(HC):
                        nc.tensor.matmul(
                            psf[:],
                            w4s[:, k, ds(m * P, P)],
                            hprev[:, k, :],
                            start=(k == 0),
                            stop=(k == HC - 1),
                        )
                        nc.tensor.matmul(
                            pst[:],
                            w4s[:, k, ds(m * P, P)],
                            dhprev[:, k, :],
                            start=(k == 0),
                            stop=(k == HC - 1),
                        )
                    nc.scalar.activation(
                        fnnT[:, m, :], psf[:], IDENT, bias=b4p[:, m : m + 1]
                    )
                    nc.scalar.copy(dfnnT[:, m, :], pst[:])

                if s + 1 < NSTRIPES:
                    pending = emit_input(s + 1)

                # ---- combine in transposed space ----
                xtT = fp.tile([P, DC, S], F32, tag="xtT")
                dtT = fp.tile([P, DC, S], F32, tag="dtT")
                for fc in range(DC):
                    tm = sp.tile([P, S], F32, tag="tmp")
                    # dt = x1 - x0 first (uses raw f32 inputs)
                    nc.vector.tensor_tensor(
                        dtT[:, fc, :], ztf1[:, fc, :], ztf0[:, fc, :], SUB
                    )
                    nc.vector.tensor_tensor(
                        xtT[:, fc, :], ztf0[:, fc, :], omt[:], MULT
                    )
                    nc.vector.tensor_tensor(tm[:], ztf1[:, fc, :], tsb[:], MULT)
                    nc.vector.tensor_tensor(xtT[:, fc, :], xtT[:, fc, :], tm[:], ADD)
                    tm2 = sp.tile([P, S], F32, tag="tmp")
                    nc.vector.tensor_tensor(tm2[:], fnnT[:, fc, :], a_[:], MULT)
                    nc.vector.tensor_tensor(xtT[:, fc, :], xtT[:, fc, :], tm2[:], ADD)
                    tm3 = sp.tile([P, S], F32, tag="tmp")
                    nc.vector.tensor_tensor(tm3[:], fnnT[:, fc, :], om2t[:], MULT)
                    nc.vector.tensor_tensor(dtT[:, fc, :], dtT[:, fc, :], tm3[:], ADD)
                    tm4 = sp.tile([P, S], F32, tag="tmp")
                    nc.vector.tensor_tensor(tm4[:], dfnnT[:, fc, :], a_[:], MULT)
                    nc.vector.tensor_tensor(dtT[:, fc, :], dtT[:, fc, :], tm4[:], ADD)

                nc.sync.dma_start(xtTv[:, :, ds(row0, S)], xtT[:])
                nc.sync.dma_start(dtTv[:, :, ds(row0, S)], dtT[:])

    nc.compile()
    return nc


def _get_nc():
    global _nc_cache
    if _nc_cache is None:
        _nc_cache = build()
    return _nc_cache


def kernel(x0, x1, t, W1, b1, W2, b2, W3, b3, W4, b4, trace=False, **trace_kwargs):
    nc = _get_nc()
    import ml_dtypes

    bf = ml_dtypes.bfloat16
    W1 = np.asarray(W1, np.float32)
    reps = {
        "W1b": np.ascontiguousarray(W1.astype(bf)),
        "w1row": np.ascontiguousarray(W1[2 * D], np.float32),
        "b1": np.ascontiguousarray(b1, np.float32),
        "W2b": np.ascontiguousarray(np.asarray(W2, np.float32).astype(bf)),
        "b2": np.ascontiguousarray(b2, np.float32),
        "W3b": np.ascontiguousarray(np.asarray(W3, np.float32).astype(bf)),
        "b3": np.ascontiguousarray(b3, np.float32),
        "W4b": np.ascontiguousarray(np.asarray(W4, np.float32).astype(bf)),
        "b4": np.ascontiguousarray(b4, np.float32),
    }
    w15b = np.zeros((P, H), dtype=bf)
    w15b[0] = W1[2 * D].astype(bf)
    reps["w15b"] = w15b
    reps["z5z"] = np.zeros((P, S), dtype=bf)
    x0 = np.asarray(x0, np.float32)
    x1 = np.asarray(x1, np.float32)
    in_maps = []
    for c in range(NCORES):
        sl = slice(c * BL, (c + 1) * BL)
        in_maps.append(
            {
                "x0T": np.ascontiguousarray(x0[sl].T),
                "x1T": np.ascontiguousarray(x1[sl].T),
                "t": np.ascontiguousarray(t[sl], np.float32),
                **reps,
            }
        )
    res = run_bass_kernel_spmd(
        nc, in_maps, list(range(NCORES)), trace=trace, **trace_kwargs
    )
    xt = np.concatenate(
        [np.ascontiguousarray(res.results[c]["xtT"].T) for c in range(NCORES)], axis=0
    )
    dt_xt = np.concatenate(
        [np.ascontiguousarray(res.results[c]["dtT"].T) for c in range(NCORES)], axis=0
    )
    if trace:
        kernel.last_result = res
    return (xt, dt_xt)
